# revision 9
# baseline (speedup 1.0000x reference)
"""Embedding lookup (one_hot(x) @ W.T + b) as a Bass/Trainium2 kernel.

Problem shapes (hardcoded; see harness contract):
    x: [16, 8192] int   (class ids < 4096)
    W: [512, 4096] f32  (nn.Linear weight; we gather rows of W.T)
    b: [512] f32
    out: [16, 8192, 512] f32 = take(W.T, x, axis=0) + b

Strategy: data-parallel over the 8 NeuronCores — each core handles 16384
tokens.  Per core, a 3-stage static pipeline:
    gpsimd.dma_gather : HBM table [4096, 512] -> SBUF tile [128, 16, 512]
                        (2048 tokens per call, 2KB per token)
    vector            : += bias (broadcast along partitions and chunks)
    sync.dma_start    : SBUF tile -> contiguous 4MB HBM block

Index slots are permuted host-side so the gather's dst layout
(dst[i%128, i//128] = token of slot i) lands tokens in blocked order:
slot i <- token (i%128)*16 + i//128, making every write-out DMA one fully
contiguous [128, 8192] f32 copy.
"""

import numpy as np

import concourse.bacc as bacc
import concourse.mybir as mybir
from concourse.bass_utils import run_bass_kernel_spmd
from concourse.library_config import mlp

N_CORES = 8
NCLS = 4096          # table rows
EMB = 512            # embedding dim (2KB rows)
TOK = 16384          # tokens per core (131072 / 8)
BLK = 2048           # tokens per dma_gather call
C = BLK // 128       # 16 chunks per partition per block
NBLK = TOK // BLK    # 8 blocks
NBUF = 4             # SBUF data tiles in flight

TRACE = False        # set by test.py to capture an NTFF profile
LAST_RESULTS = None  # BassKernelResults from the most recent run

_NCS = {}


def _build_nc(reps=1):
    nc = bacc.Bacc("TRN2", debug=False)
    f32 = mybir.dt.float32

    wt = nc.dram_tensor("wt", [NCLS, EMB], f32, kind="ExternalInput")
    bias = nc.dram_tensor("bias", [128, EMB], f32, kind="ExternalInput")
    idx = nc.dram_tensor("idx", [128, TOK // 16], mybir.dt.int16,
                         kind="ExternalInput")
    out = nc.dram_tensor("out", [TOK, EMB], f32, kind="ExternalOutput")
    # out rows in blocked order: row = j*BLK + p*C + c  <->  [j, p, c, e]
    out_v = out[:].rearrange("(j p c) e -> j p c e", p=128, c=C)

    from contextlib import ExitStack

    with (
        nc.sbuf_tensor("idx_sb", [128, TOK // 16], mybir.dt.int16) as idx_sb,
        nc.sbuf_tensor("b_sb", [128, EMB], f32) as b_sb,
        nc.semaphore("io_sem") as io_sem,
        nc.semaphore("a_sem") as a_sem,
        ExitStack() as stack,
        nc.Block() as block,
    ):
        tiles = [
            stack.enter_context(nc.sbuf_tensor(f"t{n}", [128, C, EMB], f32))
            for n in range(NBUF)
        ]
        g_sems = [stack.enter_context(nc.semaphore(f"g{j}")) for j in range(NBLK)]
        wr_sems = [stack.enter_context(nc.semaphore(f"w{j}")) for j in range(NBLK)]

        nk = reps * NBLK  # linear block index k; block j = k % NBLK

        @block.gpsimd
        def _(gp):
            gp.load_library(mlp)
            gp.dma_start(idx_sb[:], idx[:]).then_inc(io_sem, 16)
            gp.dma_start(b_sb[:], bias[:]).then_inc(io_sem, 16)
            gp.wait_ge(io_sem, 32)
            for k in range(nk):
                if k >= NBUF:
                    # tile reuse: wait until block k-NBUF left the chip
                    kp = k - NBUF
                    gp.wait_ge(wr_sems[kp % NBLK], 16 * (kp // NBLK + 1))
                gp.dma_gather(
                    tiles[k % NBUF][:],
                    wt[:],
                    idx_sb[:, (k % NBLK) * (BLK // 16):(k % NBLK + 1) * (BLK // 16)],
                    BLK,
                    BLK,
                    EMB,
                    single_packet=False,
                ).then_inc(g_sems[k % NBLK], 16)

        @block.vector
        def _(vec):
            vec.wait_ge(io_sem, 32)
            for k in range(nk):
                vec.wait_ge(g_sems[k % NBLK], 16 * (k // NBLK + 1))
                t = tiles[k % NBUF]
                vec.tensor_add(
                    t[:],
                    t[:],
                    b_sb[:, None, :].to_broadcast([128, C, EMB]),
                ).then_inc(a_sem, 1)

        @block.sync
        def _(sy):
            for k in range(nk):
                sy.wait_ge(a_sem, k + 1)
                sy.dma_start(out_v[k % NBLK], tiles[k % NBUF][:]).then_inc(
                    wr_sems[k % NBLK], 16
                )
            for j in range(NBLK):
                sy.wait_ge(wr_sems[j], 16 * reps)

    nc.compile()
    return nc


def _get_nc(reps=1):
    if reps not in _NCS:
        _NCS[reps] = _build_nc(reps)
    return _NCS[reps]


def _make_idx_input(xs):
    """Map a core's token->class array [TOK] to the int16 SBUF index layout.

    dma_gather slot i (dst partition i%128, chunk i//128) reads SBUF index
    [i%16, i//16] of its block, and we want slot i to carry token
    p*C + c (p=i%128, c=i//128) so the write-out is contiguous.
    """
    xs = xs.astype(np.int16)
    s = xs.reshape(NBLK, 128, C).transpose(0, 2, 1).reshape(NBLK, BLK)
    # wrap each block into 16 partitions: wr[p16, col] = s[col*16 + p16]
    wr = s.reshape(NBLK, BLK // 16, 16).transpose(0, 2, 1)  # [NBLK, 16, BLK//16]
    wr = np.tile(wr, (1, 8, 1))                             # [NBLK, 128, BLK//16]
    return np.ascontiguousarray(
        wr.transpose(1, 0, 2).reshape(128, TOK // 16)
    )


def kernel(x, W, b, _reps=1):
    global LAST_RESULTS
    x = np.asarray(x)
    W = np.asarray(W, dtype=np.float32)
    b = np.asarray(b, dtype=np.float32)
    batch, seq = x.shape

    xf = x.reshape(-1)
    wt = np.ascontiguousarray(W.T)                # [4096, 512]
    bias = np.ascontiguousarray(np.tile(b[None, :], (128, 1)))

    per = xf.shape[0] // N_CORES
    assert per == TOK, (xf.shape, TOK)
    in_maps = [
        {
            "wt": wt,
            "bias": bias,
            "idx": _make_idx_input(xf[c * per:(c + 1) * per]),
        }
        for c in range(N_CORES)
    ]

    nc = _get_nc(_reps)
    res = run_bass_kernel_spmd(
        nc, in_maps, core_ids=list(range(N_CORES)), trace=TRACE,
    )
    LAST_RESULTS = res

    out = np.concatenate([r["out"] for r in res.results], axis=0)
    return out.reshape(batch, seq, EMB)
